# revision 1
# baseline (speedup 1.0000x reference)
"""Trainium2 Bass kernel for nn_MOAB_46273977647401.

Network (reference):
  x1 (256,256), x3 (256,) -> 4 outer sigmoid maps (256,257,257)
  -> 1x1 conv combine (4ch) + eval BN + leaky(0.1) -> (256, 66049)
  -> FC (66049 -> 512) + relu -> FC (512 -> 4)

Sharding: 8 cores = 4 batch shards (Bc=64) x 2 H shards (Hc=256).
Per core, z maps are computed in [i-partitions, (j,b)-free] layout:
  z[i, (j,b)] = sigmoid(f(b0/b1[i], x1T[j,b]))
using PE row-broadcast of x1T into PSUM, ScalarE sigmoid with per-partition
bias/scale, DVE+Pool combine (conv+BN folded to 4 scales + offset, leaky),
feeding a K=66049 PSUM-accumulated matmul with host-relaid fc_w (i,j,h).
"""

import numpy as np

import concourse.bass as bass
import concourse.tile as tile
from concourse import bacc, mybir
from concourse.bass_utils import run_bass_kernel_spmd

F32 = mybir.dt.float32
BF16 = mybir.dt.bfloat16
AL = mybir.AluOpType

B, N, H, C = 256, 256, 512, 4
NP = 257                  # N+1
P_B, P_H = 4, 2           # batch shards x h shards
BC = B // P_B             # 64 batch rows per core
HC = H // P_H             # 256 fc outputs per core
JC = 16                   # j values per chunk (main loop, j in [1,257))
CH = JC * BC              # 1024 free elems per chunk
NCHUNK = 256 // JC        # 16
EPS = 1e-10
BN_EPS = 1e-5
LEAKY = 0.1

# W dtype for the big fc_w stream + matmul lhs (y) dtype.
W_DTYPE = BF16
W_NP = np.dtype(np.float32) if W_DTYPE == F32 else np.dtype("bfloat16")


def build_program():
    nc = bacc.Bacc("TRN2", target_bir_lowering=False, debug=False, num_devices=8)

    d_a0T = nc.dram_tensor("a0T", [NP, BC], F32, kind="ExternalInput").ap()
    d_a1T = nc.dram_tensor("a1T", [NP, BC], F32, kind="ExternalInput").ap()
    d_aflat = nc.dram_tensor("aflat", [NCHUNK, CH], F32, kind="ExternalInput").ap()
    d_b0 = nc.dram_tensor("b0", [NP, 1], F32, kind="ExternalInput").ap()
    d_b1 = nc.dram_tensor("b1", [NP, 1], F32, kind="ExternalInput").ap()
    d_cv = nc.dram_tensor("cv", [128, 1], F32, kind="ExternalInput").ap()
    d_sv = nc.dram_tensor("sv", [128, 6], F32, kind="ExternalInput").ap()
    d_w3 = nc.dram_tensor("w3", [NP, NP, HC], W_DTYPE, kind="ExternalInput").ap()
    d_wstrip = nc.dram_tensor("wstrip", [NP, HC], W_DTYPE, kind="ExternalInput").ap()
    d_wcol0 = nc.dram_tensor("wcol0", [NP, HC], W_DTYPE, kind="ExternalInput").ap()
    d_fcb = nc.dram_tensor("fcb", [HC, 1], F32, kind="ExternalInput").ap()
    d_owt = nc.dram_tensor("owt", [HC, C], F32, kind="ExternalInput").ap()
    d_eye = nc.dram_tensor("eye", [64, 64], F32, kind="ExternalInput").ap()
    d_out = nc.dram_tensor("out", [BC, C], F32, kind="ExternalOutput").ap()

    with tile.TileContext(nc) as tc:
        with (
            tc.tile_pool(name="const", bufs=1) as cpool,
            tc.tile_pool(name="setup", bufs=1) as spool,
            tc.tile_pool(name="stage", bufs=3) as stpool,
            tc.tile_pool(name="w0", bufs=3) as wpool0,
            tc.tile_pool(name="w1", bufs=3) as wpool1,
            tc.tile_pool(name="z", bufs=3) as zpool,
            tc.tile_pool(name="comb", bufs=3) as combpool,
            tc.tile_pool(name="ypool", bufs=3) as ypool,
            tc.tile_pool(name="fin", bufs=1) as finpool,
            tc.tile_pool(name="psA", bufs=2, space="PSUM") as psA,
            tc.tile_pool(name="psR", bufs=1, space="PSUM") as psR,
            tc.tile_pool(name="psO", bufs=1, space="PSUM") as psO,
            tc.tile_pool(name="psT", bufs=1, space="PSUM") as psT,
        ):
            # ---------------- constants / setup ----------------
            a0 = [cpool.tile([128, BC], F32, tag=f"a0_{k}", name=f"a0_{k}") for k in range(3)]
            a1 = [cpool.tile([128, BC], F32, tag=f"a1_{k}", name=f"a1_{k}") for k in range(3)]
            nc.sync.dma_start(a0[0][:, :], d_a0T[0:128, :])
            nc.sync.dma_start(a0[1][:, :], d_a0T[128:256, :])
            nc.sync.dma_start(a0[2][0:1, :], d_a0T[256:257, :])
            nc.sync.dma_start(a1[0][:, :], d_a1T[0:128, :])
            nc.sync.dma_start(a1[1][:, :], d_a1T[128:256, :])
            nc.sync.dma_start(a1[2][0:1, :], d_a1T[256:257, :])

            b0t = [cpool.tile([128, 1], F32, tag=f"b0_{k}", name=f"b0_{k}") for k in range(2)]
            b1t = [cpool.tile([128, 1], F32, tag=f"b1_{k}", name=f"b1_{k}") for k in range(2)]
            nc.sync.dma_start(b0t[0][:, :], d_b0[0:128, :])
            nc.sync.dma_start(b0t[1][:, :], d_b0[128:256, :])
            nc.sync.dma_start(b1t[0][:, :], d_b1[0:128, :])
            nc.sync.dma_start(b1t[1][:, :], d_b1[128:256, :])

            cv = cpool.tile([128, 1], F32, tag="cv")
            sv = cpool.tile([128, 6], F32, tag="sv")
            nc.sync.dma_start(cv[:, :], d_cv[:, :])
            nc.sync.dma_start(sv[:, :], d_sv[:, :])

            fcb = [cpool.tile([128, 1], F32, tag=f"fcb_{k}", name=f"fcb_{k}") for k in range(2)]
            owt = [cpool.tile([128, C], F32, tag=f"owt_{k}", name=f"owt_{k}") for k in range(2)]
            nc.sync.dma_start(fcb[0][:, :], d_fcb[0:128, :])
            nc.sync.dma_start(fcb[1][:, :], d_fcb[128:256, :])
            nc.sync.dma_start(owt[0][:, :], d_owt[0:128, :])
            nc.sync.dma_start(owt[1][:, :], d_owt[128:256, :])

            eye = cpool.tile([64, 64], F32, tag="eye")
            nc.sync.dma_start(eye[:, :], d_eye[:, :])

            ones1 = cpool.tile([1, 128], F32, tag="ones1")
            nc.vector.memset(ones1[:, :], 1.0)

            # recip tiles for the i=256 strip: r = 1/(a1 + eps), (j,b) layout
            rt = [cpool.tile([128, BC], F32, tag=f"r_{k}", name=f"r_{k}") for k in range(3)]
            for k, npart in ((0, 128), (1, 128), (2, 1)):
                tmp = spool.tile([128, BC], F32, tag=f"rtmp_{k}")
                nc.vector.tensor_scalar_add(
                    tmp[0:npart, :], a1[k][0:npart, :], EPS
                )
                nc.vector.reciprocal(rt[k][0:npart, :], tmp[0:npart, :])

            # rflat16: recip of aflat rows, in [NCHUNK, CH] layout (on 16
            # partitions; only used as DMA source for per-chunk staging)
            af16 = spool.tile([NCHUNK, CH], F32, tag="af16")
            nc.sync.dma_start(af16[:, :], d_aflat[:, :])
            rf16 = cpool.tile([NCHUNK, CH], F32, tag="rf16")
            rtmp16 = spool.tile([NCHUNK, CH], F32, tag="rtmp16")
            nc.vector.tensor_scalar_add(rtmp16[:, :], af16[:, :], EPS)
            nc.vector.reciprocal(rf16[:, :], rtmp16[:, :])

            # ---------------- main accumulation ----------------
            psum_out = psO.tile([BC, HC], F32, tag="acc")
            mm_started = [False]

            def acc_mm(lhsT, rhs, stop=False):
                nc.tensor.matmul(
                    psum_out[:, :],
                    lhsT,
                    rhs,
                    start=not mm_started[0],
                    stop=stop,
                    skip_group_check=True,
                )
                mm_started[0] = True

            for c in range(NCHUNK):
                j0 = 1 + c * JC
                # stage a/r rows for this chunk on partition 0
                stg_a = stpool.tile([1, CH], F32, tag="stg_a")
                nc.sync.dma_start(stg_a[:, :], d_aflat[c : c + 1, :])
                stg_r = stpool.tile([1, CH], F32, tag="stg_r")
                nc.gpsimd.dma_start(stg_r[:, :], rf16[c : c + 1, :])

                # PE broadcast to 128 partitions (PSUM)
                arep = psA.tile([128, CH], F32, tag="arep")
                rrep = psR.tile([128, CH], F32, tag="rrep")
                for half in range(2):
                    sl = slice(half * 512, (half + 1) * 512)
                    nc.tensor.matmul(
                        arep[:, sl], ones1[:, :], stg_a[:, sl],
                        start=True, stop=True, skip_group_check=True,
                    )
                    nc.tensor.matmul(
                        rrep[:, sl], ones1[:, :], stg_r[:, sl],
                        start=True, stop=True, skip_group_check=True,
                    )

                # W slabs for both i-tiles
                wsl = []
                for it, wpool in ((0, wpool0), (1, wpool1)):
                    w = wpool.tile([128, JC * HC], W_DTYPE, tag=f"wsl{it}")
                    nc.sync.dma_start(
                        w[:, :], d_w3[it * 128 : (it + 1) * 128, j0 : j0 + JC, :]
                    )
                    wsl.append(w)

                for it in range(2):
                    SIG = mybir.ActivationFunctionType.Sigmoid
                    za = zpool.tile([128, CH], BF16, tag="za")
                    nc.scalar.activation(za[:, :], arep[:, :], SIG,
                                         bias=b0t[it][:, :], scale=1.0)
                    zs = zpool.tile([128, CH], BF16, tag="zs")
                    nc.scalar.activation(zs[:, :], arep[:, :], SIG,
                                         bias=b0t[it][:, :], scale=-1.0)
                    zp = zpool.tile([128, CH], BF16, tag="zp")
                    nc.scalar.activation(zp[:, :], arep[:, :], SIG,
                                         bias=0.0, scale=b1t[it][:, :])
                    zd = zpool.tile([128, CH], BF16, tag="zd")
                    nc.scalar.activation(zd[:, :], rrep[:, :], SIG,
                                         bias=0.0, scale=b1t[it][:, :])

                    # y = s0*za + s1*zs + s2*zp + s3*zd + off ; leaky
                    # all-bf16 DVE chain: ts gets 4x mode, stt gets 2x
                    # ts (4x bf16) scale passes + tt (2x bf16) adds beat
                    # stt chains (stt has no bf16 2x uop -> 1x)
                    ta = combpool.tile([128, CH], BF16, tag="ta")
                    nc.vector.tensor_scalar(ta[:, :], za[:, :],
                                            sv[:, 0:1], sv[:, 4:5],
                                            AL.mult, AL.add)
                    tb = combpool.tile([128, CH], BF16, tag="tb")
                    nc.vector.tensor_scalar(tb[:, :], zs[:, :],
                                            sv[:, 1:2], None, AL.mult)
                    tc2 = combpool.tile([128, CH], BF16, tag="tc2")
                    nc.vector.tensor_scalar(tc2[:, :], zp[:, :],
                                            sv[:, 2:3], None, AL.mult)
                    td = combpool.tile([128, CH], BF16, tag="td")
                    nc.vector.tensor_scalar(td[:, :], zd[:, :],
                                            sv[:, 3:4], None, AL.mult)
                    u1 = combpool.tile([128, CH], BF16, tag="u1")
                    nc.vector.tensor_add(u1[:, :], ta[:, :], tb[:, :])
                    u2 = combpool.tile([128, CH], BF16, tag="u2")
                    nc.vector.tensor_add(u2[:, :], tc2[:, :], td[:, :])
                    y1 = combpool.tile([128, CH], BF16, tag="y1")
                    nc.vector.tensor_add(y1[:, :], u1[:, :], u2[:, :])
                    lk = combpool.tile([128, CH], BF16, tag="lk")
                    nc.vector.tensor_scalar(lk[:, :], y1[:, :],
                                            LEAKY, None, AL.mult)
                    yl = ypool.tile([128, CH], W_DTYPE, tag="yl")
                    nc.vector.tensor_tensor(yl[:, :], y1[:, :], lk[:, :],
                                            AL.max)

                    for jw in range(JC):
                        acc_mm(
                            yl[:, jw * BC : (jw + 1) * BC],
                            wsl[it][:, jw * HC : (jw + 1) * HC],
                        )

            # ---------------- strip j=0 (i in [0,256)) ----------------
            SIG = mybir.ActivationFunctionType.Sigmoid
            for it in range(2):
                za0 = spool.tile([128, 1], F32, tag=f"za0_{it}")
                nc.scalar.activation(za0[:, :], b0t[it][:, :], SIG)
                zp0 = spool.tile([128, 1], F32, tag=f"zp0_{it}")
                nc.scalar.activation(zp0[:, :], b1t[it][:, :], SIG)
                zd0 = spool.tile([128, 1], F32, tag=f"zd0_{it}")
                nc.scalar.activation(zd0[:, :], b1t[it][:, :], SIG,
                                     bias=0.0, scale=1.0 / (1.0 + EPS))
                tt = spool.tile([128, 1], F32, tag=f"tt0_{it}")
                nc.vector.tensor_scalar(tt[:, :], za0[:, :],
                                        sv[:, 5:6], sv[:, 4:5],
                                        AL.mult, AL.add)
                nc.vector.scalar_tensor_tensor(tt[:, :], zp0[:, :],
                                               sv[:, 2:3], tt[:, :],
                                               AL.mult, AL.add)
                nc.vector.scalar_tensor_tensor(tt[:, :], zd0[:, :],
                                               sv[:, 3:4], tt[:, :],
                                               AL.mult, AL.add)
                yl0 = spool.tile([128, 1], F32, tag=f"yl0_{it}")
                nc.vector.scalar_tensor_tensor(yl0[:, :], tt[:, :],
                                               LEAKY, tt[:, :],
                                               AL.mult, AL.max)
                yj0 = spool.tile([128, BC], W_DTYPE, tag=f"yj0_{it}")
                nc.vector.tensor_copy(yj0[:, :],
                                      yl0[:, 0:1].broadcast_to([128, BC]))
                wj0 = spool.tile([128, HC], W_DTYPE, tag=f"wj0_{it}")
                nc.sync.dma_start(wj0[:, :],
                                  d_wcol0[it * 128 : (it + 1) * 128, :])
                acc_mm(yj0[:, :], wj0[:, :])

            # ---------------- strip i=256 (j in [0,257)) ----------------
            for jt, (jof, jsz) in enumerate(((0, 128), (128, 128), (256, 1))):
                za = spool.tile([128, BC], F32, tag=f"sza_{jt}")
                nc.scalar.activation(za[0:jsz, :], a0[jt][0:jsz, :], SIG,
                                     bias=cv[0:jsz, :], scale=1.0)
                zs = spool.tile([128, BC], F32, tag=f"szs_{jt}")
                nc.scalar.activation(zs[0:jsz, :], a0[jt][0:jsz, :], SIG,
                                     bias=cv[0:jsz, :], scale=-1.0)
                zp = spool.tile([128, BC], F32, tag=f"szp_{jt}")
                nc.scalar.activation(zp[0:jsz, :], a1[jt][0:jsz, :], SIG,
                                     bias=0.0, scale=cv[0:jsz, :])
                zd = spool.tile([128, BC], F32, tag=f"szd_{jt}")
                nc.scalar.activation(zd[0:jsz, :], rt[jt][0:jsz, :], SIG,
                                     bias=0.0, scale=cv[0:jsz, :])
                t1 = spool.tile([128, BC], F32, tag=f"st1_{jt}")
                nc.vector.tensor_scalar(t1[0:jsz, :], za[0:jsz, :],
                                        sv[0:jsz, 0:1], sv[0:jsz, 4:5],
                                        AL.mult, AL.add)
                nc.vector.scalar_tensor_tensor(t1[0:jsz, :], zs[0:jsz, :],
                                               sv[0:jsz, 1:2], t1[0:jsz, :],
                                               AL.mult, AL.add)
                nc.vector.scalar_tensor_tensor(t1[0:jsz, :], zp[0:jsz, :],
                                               sv[0:jsz, 2:3], t1[0:jsz, :],
                                               AL.mult, AL.add)
                nc.vector.scalar_tensor_tensor(t1[0:jsz, :], zd[0:jsz, :],
                                               sv[0:jsz, 3:4], t1[0:jsz, :],
                                               AL.mult, AL.add)
                yls = spool.tile([128, BC], W_DTYPE, tag=f"syl_{jt}")
                nc.vector.scalar_tensor_tensor(yls[0:jsz, :], t1[0:jsz, :],
                                               LEAKY, t1[0:jsz, :],
                                               AL.mult, AL.max)
                ws = spool.tile([128, HC], W_DTYPE, tag=f"sws_{jt}")
                nc.sync.dma_start(ws[0:jsz, :], d_wstrip[jof : jof + jsz, :])
                acc_mm(yls[0:jsz, :], ws[0:jsz, :], stop=(jt == 2))

            # ---------------- tail: relu + fc2 ----------------
            y2 = finpool.tile([BC, HC], F32, tag="y2")
            nc.scalar.copy(y2[:, :], psum_out[:, :])
            lg = psO.tile([BC, C], F32, tag="acc")
            for h2 in range(2):
                yT = psT.tile([128, BC], F32, tag="yT")
                nc.tensor.transpose(yT[:, :],
                                    y2[:, h2 * 128 : (h2 + 1) * 128],
                                    eye[:, :])
                ryT = finpool.tile([128, BC], F32, tag=f"ryT_{h2}")
                nc.scalar.activation(ryT[:, :], yT[:, :],
                                     mybir.ActivationFunctionType.Relu,
                                     bias=fcb[h2][:, :], scale=1.0)
                nc.tensor.matmul(lg[:, :], ryT[:, :], owt[h2][:, :],
                                 start=(h2 == 0), stop=(h2 == 1),
                                 skip_group_check=True)
            outt = finpool.tile([BC, C], F32, tag="outt")
            nc.scalar.copy(outt[:, :], lg[:, :])
            nc.sync.dma_start(d_out[:, :], outt[:, :])

    nc.finalize()
    return nc


_CACHED_NC = None


def _get_program():
    global _CACHED_NC
    if _CACHED_NC is None:
        _CACHED_NC = build_program()
    return _CACHED_NC


def make_in_maps(x1, x3, conv_w, conv_b, bn_gamma, bn_beta, bn_mean, bn_var,
                 fc_w, fc_b, out_w, out_b):
    x1 = np.asarray(x1, np.float32)
    x3 = np.asarray(x3, np.float32)
    fc_w = np.asarray(fc_w, np.float32)

    g = float(np.asarray(bn_gamma).reshape(-1)[0]) / float(
        np.sqrt(np.asarray(bn_var).reshape(-1)[0] + BN_EPS))
    s = np.asarray(conv_w, np.float32).reshape(-1) * g
    off = (float(np.asarray(conv_b).reshape(-1)[0])
           - float(np.asarray(bn_mean).reshape(-1)[0])) * g \
        + float(np.asarray(bn_beta).reshape(-1)[0])

    sv = np.zeros((128, 6), np.float32)
    sv[:, 0], sv[:, 1], sv[:, 2], sv[:, 3] = s[0], s[1], s[2], s[3]
    sv[:, 4] = off
    sv[:, 5] = s[0] + s[1]
    cv = np.full((128, 1), x3[-1], np.float32)

    b0 = np.concatenate([[0.0], x3]).astype(np.float32).reshape(NP, 1)
    b1 = np.concatenate([[1.0], x3]).astype(np.float32).reshape(NP, 1)

    # fc_w (H, 66049) with k = i*257+j  ->  W3 [i, j, h]
    w3 = np.ascontiguousarray(
        fc_w.reshape(H, NP, NP).transpose(1, 2, 0))
    eye = np.eye(64, dtype=np.float32)

    x1T = np.ascontiguousarray(x1.T)  # (256 j, 256 b)

    in_maps = []
    for core in range(8):
        bp, hq = core // P_H, core % P_H
        xs = np.ascontiguousarray(x1T[:, bp * BC : (bp + 1) * BC])
        a0T = np.concatenate([np.zeros((1, BC), np.float32), xs])
        a1T = np.concatenate([np.ones((1, BC), np.float32), xs])
        aflat = np.ascontiguousarray(xs.reshape(NCHUNK, CH))
        hsl = slice(hq * HC, (hq + 1) * HC)
        w3q = np.ascontiguousarray(w3[:, :, hsl]).astype(W_NP)
        in_maps.append({
            "a0T": a0T, "a1T": a1T, "aflat": aflat,
            "b0": b0, "b1": b1, "cv": cv, "sv": sv,
            "w3": w3q,
            "wstrip": np.ascontiguousarray(w3q[256, :, :]),
            "wcol0": np.ascontiguousarray(w3q[:, 0, :]),
            "fcb": np.asarray(fc_b, np.float32)[hsl].reshape(HC, 1),
            "owt": np.ascontiguousarray(
                np.asarray(out_w, np.float32)[:, hsl].T),
            "eye": eye,
        })
    return in_maps


def kernel(**inputs):
    in_maps = make_in_maps(**inputs)
    nc = _get_program()
    res = run_bass_kernel_spmd(nc, in_maps, list(range(8)))

    out = np.zeros((B, C), np.float32)
    outb = np.asarray(inputs["out_b"], np.float32).reshape(1, C)
    for bp in range(P_B):
        acc = np.zeros((BC, C), np.float32)
        for hq in range(P_H):
            acc += res.results[bp * P_H + hq]["out"]
        out[bp * BC : (bp + 1) * BC] = acc + outb
    return out

